# revision 1
# baseline (speedup 1.0000x reference)
"""Trainium2 Bass kernel for nn_ConvblockNofrills (dense_cnn).

Reference computation (per batch b, output position l, channel d):
    gate[b,l,d,k] = tanh( sum_c x[b, l+K-1, c] * weights[d, c, k] )
    out[b,l,d]    = sum_k x[b, l+k, d] * gate[b,l,d,k]
with B=8, T=4096, C=D=512, K=7, L=T-K+1=4090.

Strategy: data-parallel across the 8 NeuronCores (one batch each).
Per core everything runs in transposed (channel, position) layout:
  - gates via bf16 matmul on TensorE: 28 (k, d-chunk) weight tiles x
    4 c-chunks x 8 l-tiles of 512, accumulated in PSUM (fp32)
  - tanh on ScalarE (fp32-accurate spline), output bf16 to SBUF
  - 7-tap multiply/accumulate on VectorE in bf16 (2x packed mode; a
    1-element-shifted copy of x keeps every slice 4-byte aligned)
Host side transposes/casts inputs (part of sharding) and transposes the
(C, L) bf16 per-core result back to the (B, L, C) fp32 output.
"""

import numpy as np
import ml_dtypes

import sys
for _p in ("/opt/trn_rl_repo", "/root/.axon_site/_ro/trn_rl_repo"):
    if _p not in sys.path:
        sys.path.append(_p)

B, T, C, K = 8, 4096, 512, 7
L = T - K + 1  # 4090
NCORES = 8
P = 128           # partitions
DC = C // P       # 4 channel chunks
NL = 512          # l-tile (one PSUM bank of fp32)
NLT = (L + NL - 1) // NL  # 8 l-tiles, last ragged (506)

_cache = {}


def _build():
    import concourse.bass as bass  # noqa: F401
    import concourse.mybir as mybir
    import concourse.tile as tile
    from concourse import bacc

    bf16 = mybir.dt.bfloat16
    f32 = mybir.dt.float32
    Tanh = mybir.ActivationFunctionType.Tanh

    nc = bacc.Bacc("TRN2", target_bir_lowering=False, debug=False,
                   num_devices=NCORES)

    xT_d = nc.dram_tensor("xT", [C, T], bf16, kind="ExternalInput")
    xTo_d = nc.dram_tensor("xTo", [C, T], bf16, kind="ExternalInput")
    wT_d = nc.dram_tensor("wT", [K, C, C], bf16, kind="ExternalInput")
    outT_d = nc.dram_tensor("outT", [C, L], bf16, kind="ExternalOutput")

    with tile.TileContext(nc) as tc:
        with (
            tc.tile_pool(name="wpool", bufs=1) as wpool,
            tc.tile_pool(name="xpool", bufs=1) as xpool,
            tc.tile_pool(name="gpool", bufs=3) as gpool,
            tc.tile_pool(name="apool", bufs=3) as apool,
            tc.tile_pool(name="ppool", bufs=2) as ppool,
            tc.tile_pool(name="psum", bufs=8, space="PSUM") as psum_pool,
        ):
            # Resident weights, one tile per contraction chunk:
            # w_sb[cc][c_in, k, d] = weights[d, cc*128+c_in, k]
            w_sb = []
            for cc in range(DC):
                w = wpool.tile([P, K, C], bf16, name=f"w_{cc}")
                src = wT_d.ap()[:, cc * P:(cc + 1) * P, :].rearrange(
                    "k c d -> c k d")
                nc.sync.dma_start(w[:], src)
                w_sb.append(w)

            # Resident transposed activations (and 1-shifted copy for
            # 4B-aligned odd-k slices).
            xT_sb, xTo_sb = [], []
            for cc in range(DC):
                xt = xpool.tile([P, T], bf16, name=f"xt_{cc}")
                nc.sync.dma_start(xt[:], xT_d.ap()[cc * P:(cc + 1) * P, :])
                xT_sb.append(xt)
                xto = xpool.tile([P, T], bf16, name=f"xto_{cc}")
                nc.sync.dma_start(xto[:], xTo_d.ap()[cc * P:(cc + 1) * P, :])
                xTo_sb.append(xto)

            for dc in range(DC):
                acc = None
                for k in range(K):
                    # gates for this (dc, k) over all l: PSUM accumulate
                    # over the 4 c-chunks, 8 l-tiles of 512.
                    ps = [psum_pool.tile([P, NL], f32, name=f"ps_{dc}_{k}_{lt}",
                                         tag="ps") for lt in range(NLT)]
                    for cc in range(DC):
                        lhsT = w_sb[cc][:, k, dc * P:(dc + 1) * P]
                        for lt in range(NLT):
                            l0 = lt * NL
                            nl = min(NL, L - l0)
                            nc.tensor.matmul(
                                ps[lt][:, :nl],
                                lhsT,
                                xT_sb[cc][:, l0 + K - 1: l0 + K - 1 + nl],
                                start=(cc == 0),
                                stop=(cc == DC - 1),
                            )
                    g = gpool.tile([P, L], bf16, tag="g", name=f"g_{dc}_{k}")
                    for lt in range(NLT):
                        l0 = lt * NL
                        nl = min(NL, L - l0)
                        nc.scalar.activation(g[:, l0:l0 + nl], ps[lt][:, :nl],
                                             Tanh)
                    # xu_k = xT[dc][:, k : k+L]  (shifted copy for odd k)
                    if k % 2 == 0:
                        xu = xT_sb[dc][:, k:k + L]
                    else:
                        xu = xTo_sb[dc][:, k - 1:k - 1 + L]
                    if acc is None:
                        acc = apool.tile([P, L], bf16, tag="acc",
                                         name=f"acc_{dc}_{k}")
                        nc.vector.tensor_mul(acc[:], g[:], xu)
                    else:
                        prod = ppool.tile([P, L], bf16, tag="prod",
                                          name=f"prod_{dc}_{k}")
                        nc.vector.tensor_mul(prod[:], g[:], xu)
                        nxt = apool.tile([P, L], bf16, tag="acc",
                                         name=f"accn_{dc}_{k}")
                        nc.vector.tensor_add(nxt[:], acc[:], prod[:])
                        acc = nxt
                nc.sync.dma_start(outT_d.ap()[dc * P:(dc + 1) * P, :], acc[:])

    nc.compile()
    return nc


def _prep_inputs(x, weights):
    bf = ml_dtypes.bfloat16
    wT = np.transpose(weights, (2, 1, 0)).astype(bf)  # (K, C, D)
    wT = np.ascontiguousarray(wT)
    in_maps = []
    for b in range(B):
        xT = x[b].T.astype(bf)  # (C, T) contiguous
        xTo = np.zeros_like(xT)
        xTo[:, :-1] = xT[:, 1:]
        in_maps.append({"xT": xT, "xTo": xTo, "wT": wT})
    return in_maps


def kernel(x, weights):
    x = np.asarray(x, dtype=np.float32)
    weights = np.asarray(weights, dtype=np.float32)
    assert x.shape == (B, T, C) and weights.shape == (C, C, K)

    from concourse.bass_utils import run_bass_kernel_spmd

    if "nc" not in _cache:
        _cache["nc"] = _build()
    nc = _cache["nc"]

    in_maps = _prep_inputs(x, weights)
    res = run_bass_kernel_spmd(nc, in_maps, list(range(NCORES)))

    out = np.empty((B, L, C), dtype=np.float32)
    for b in range(B):
        out[b] = res.results[b]["outT"].astype(np.float32).T
    return out


if __name__ == "__main__":
    rng = np.random.default_rng(0)
    x = rng.standard_normal((B, T, C), dtype=np.float32)
    w = (rng.standard_normal((C, C, K), dtype=np.float32)
         / np.sqrt(np.float32(C * K)))
    out = kernel(x, w)
    print("out", out.shape, out.dtype, float(np.abs(out).max()))


# revision 2
# speedup vs baseline: 1.0149x; 1.0149x over previous
"""Trainium2 Bass kernel for nn_ConvblockNofrills (dense_cnn).

Reference computation (per batch b, output position l, channel d):
    gate[b,l,d,k] = tanh( sum_c x[b, l+K-1, c] * weights[d, c, k] )
    out[b,l,d]    = sum_k x[b, l+k, d] * gate[b,l,d,k]
with B=8, T=4096, C=D=512, K=7, L=T-K+1=4090.

Strategy: data-parallel across the 8 NeuronCores (one batch each).
Per core everything runs in transposed (channel, position) layout:
  - gates via bf16 matmul on TensorE (fp32 PSUM accumulation)
  - tanh on ScalarE (fp32-accurate spline), output bf16 to SBUF
  - 7-tap multiply/accumulate on VectorE in bf16 (2x packed mode; a
    1-element-shifted copy of x keeps every slice 4-byte aligned)
Loop order (lq, dc, k, cc, lt) with position-chunked DMA loads so the
first matmuls only wait on ~2.5MB of input, and the epilogue after the
last matmul is only the final quad's tanh/multiply-add/store.
Host side transposes/casts inputs (part of sharding) and transposes the
(C, L) bf16 per-core result back to the (B, L, C) fp32 output.
"""

import numpy as np
import ml_dtypes

import sys
for _p in ("/opt/trn_rl_repo", "/root/.axon_site/_ro/trn_rl_repo"):
    if _p not in sys.path:
        sys.path.append(_p)

B, T, C, K = 8, 4096, 512, 7
L = T - K + 1  # 4090
NCORES = 8
P = 128           # partitions
DC = C // P       # 4 channel chunks
NL = 512          # l-tile (one PSUM bank of fp32)
NLT = (L + NL - 1) // NL  # 8 l-tiles, last ragged (506)
QUAD = 4          # l-tiles per group
NQ = NLT // QUAD  # 2 groups

_cache = {}


def _build():
    import concourse.bass as bass  # noqa: F401
    import concourse.mybir as mybir
    import concourse.tile as tile
    from concourse import bacc

    bf16 = mybir.dt.bfloat16
    f32 = mybir.dt.float32
    Tanh = mybir.ActivationFunctionType.Tanh

    nc = bacc.Bacc("TRN2", target_bir_lowering=False, debug=False,
                   num_devices=NCORES)

    xT_d = nc.dram_tensor("xT", [C, T], bf16, kind="ExternalInput")
    xTo_d = nc.dram_tensor("xTo", [C, T], bf16, kind="ExternalInput")
    wT_d = nc.dram_tensor("wT", [K, C, C], bf16, kind="ExternalInput")
    outT_d = nc.dram_tensor("outT", [C, L], bf16, kind="ExternalOutput")

    with tile.TileContext(nc) as tc:
        with (
            tc.tile_pool(name="wpool", bufs=1) as wpool,
            tc.tile_pool(name="xpool", bufs=1) as xpool,
            tc.tile_pool(name="gpool", bufs=6) as gpool,
            tc.tile_pool(name="apool", bufs=3) as apool,
            tc.tile_pool(name="ppool", bufs=3) as ppool,
            tc.tile_pool(name="psum", bufs=8, space="PSUM") as psum_pool,
        ):
            # Resident weights, one tile per contraction chunk:
            # w_sb[cc][c_in, k, d] = weights[d, cc*128+c_in, k]
            # Loaded per-k (k=0 first) so the first matmuls wait on ~0.5MB.
            w_sb = [wpool.tile([P, K, C], bf16, name=f"w_{cc}")
                    for cc in range(DC)]
            # Resident transposed activations + 1-col-shifted copy, loaded
            # in position blocks so compute can start after block 0.
            xT_sb = [xpool.tile([P, T], bf16, name=f"xt_{cc}")
                     for cc in range(DC)]
            xTo_sb = [xpool.tile([P, T], bf16, name=f"xto_{cc}")
                      for cc in range(DC)]

            XBLK = 1024
            for k in range(K):
                for cc in range(DC):
                    src = wT_d.ap()[k, cc * P:(cc + 1) * P, :]
                    nc.sync.dma_start(w_sb[cc][:, k, :], src)
                if k == 0:
                    # first position block of x right after the k=0 weights
                    for cc in range(DC):
                        nc.sync.dma_start(
                            xT_sb[cc][:, 0:XBLK],
                            xT_d.ap()[cc * P:(cc + 1) * P, 0:XBLK])
            for blk in range(1, T // XBLK):
                c0 = blk * XBLK
                for cc in range(DC):
                    nc.sync.dma_start(
                        xT_sb[cc][:, c0:c0 + XBLK],
                        xT_d.ap()[cc * P:(cc + 1) * P, c0:c0 + XBLK])
            for blk in range(T // XBLK):
                c0 = blk * XBLK
                for cc in range(DC):
                    nc.sync.dma_start(
                        xTo_sb[cc][:, c0:c0 + XBLK],
                        xTo_d.ap()[cc * P:(cc + 1) * P, c0:c0 + XBLK])

            for lq in range(NQ):
                lt0 = lq * QUAD
                q0 = lt0 * NL                      # first output col
                qn = min(QUAD * NL, L - q0)        # 2048 / 2042
                for dc in range(DC):
                    acc = None
                    for k in range(K):
                        ps = [psum_pool.tile([P, NL], f32, tag="ps",
                                             name=f"ps_{lq}_{dc}_{k}_{i}")
                              for i in range(QUAD)]
                        for cc in range(DC):
                            lhsT = w_sb[cc][:, k, dc * P:(dc + 1) * P]
                            for i in range(QUAD):
                                l0 = q0 + i * NL
                                nl = min(NL, L - l0)
                                nc.tensor.matmul(
                                    ps[i][:, :nl],
                                    lhsT,
                                    xT_sb[cc][:, l0 + K - 1: l0 + K - 1 + nl],
                                    start=(cc == 0),
                                    stop=(cc == DC - 1),
                                )
                        g = gpool.tile([P, QUAD * NL], bf16, tag="g",
                                       name=f"g_{lq}_{dc}_{k}")
                        for i in range(QUAD):
                            l0 = q0 + i * NL
                            nl = min(NL, L - l0)
                            nc.scalar.activation(
                                g[:, i * NL:i * NL + nl], ps[i][:, :nl], Tanh)
                        # xu_k = xT[dc][:, q0+k : q0+k+qn] (shifted for odd k)
                        if k % 2 == 0:
                            xu = xT_sb[dc][:, q0 + k:q0 + k + qn]
                        else:
                            xu = xTo_sb[dc][:, q0 + k - 1:q0 + k - 1 + qn]
                        if acc is None:
                            acc = apool.tile([P, QUAD * NL], bf16, tag="acc",
                                             name=f"acc_{lq}_{dc}_{k}")
                            nc.vector.tensor_mul(acc[:, :qn], g[:, :qn], xu)
                        else:
                            prod = ppool.tile([P, QUAD * NL], bf16,
                                              tag="prod",
                                              name=f"prod_{lq}_{dc}_{k}")
                            nc.vector.tensor_mul(prod[:, :qn], g[:, :qn], xu)
                            nxt = apool.tile([P, QUAD * NL], bf16, tag="acc",
                                             name=f"accn_{lq}_{dc}_{k}")
                            nc.vector.tensor_add(nxt[:, :qn], acc[:, :qn],
                                                 prod[:, :qn])
                            acc = nxt
                    nc.sync.dma_start(
                        outT_d.ap()[dc * P:(dc + 1) * P, q0:q0 + qn],
                        acc[:, :qn])

    nc.compile()
    return nc


def _prep_inputs(x, weights):
    bf = ml_dtypes.bfloat16
    wT = np.transpose(weights, (2, 1, 0)).astype(bf)  # (K, C, D)
    wT = np.ascontiguousarray(wT)
    in_maps = []
    for b in range(B):
        xT = x[b].T.astype(bf)  # (C, T) contiguous
        xTo = np.zeros_like(xT)
        xTo[:, :-1] = xT[:, 1:]
        in_maps.append({"xT": xT, "xTo": xTo, "wT": wT})
    return in_maps


def kernel(x, weights):
    x = np.asarray(x, dtype=np.float32)
    weights = np.asarray(weights, dtype=np.float32)
    assert x.shape == (B, T, C) and weights.shape == (C, C, K)

    from concourse.bass_utils import run_bass_kernel_spmd

    if "nc" not in _cache:
        _cache["nc"] = _build()
    nc = _cache["nc"]

    in_maps = _prep_inputs(x, weights)
    res = run_bass_kernel_spmd(nc, in_maps, list(range(NCORES)))

    out = np.empty((B, L, C), dtype=np.float32)
    for b in range(B):
        out[b] = res.results[b]["outT"].astype(np.float32).T
    return out


if __name__ == "__main__":
    rng = np.random.default_rng(0)
    x = rng.standard_normal((B, T, C), dtype=np.float32)
    w = (rng.standard_normal((C, C, K), dtype=np.float32)
         / np.sqrt(np.float32(C * K)))
    out = kernel(x, w)
    print("out", out.shape, out.dtype, float(np.abs(out).max()))


# revision 3
# speedup vs baseline: 1.0869x; 1.0709x over previous
"""Trainium2 Bass kernel for nn_ConvblockNofrills (dense_cnn).

Reference computation (per batch b, output position l, channel d):
    gate[b,l,d,k] = tanh( sum_c x[b, l+K-1, c] * weights[d, c, k] )
    out[b,l,d]    = sum_k x[b, l+k, d] * gate[b,l,d,k]
with B=8, T=4096, C=D=512, K=7, L=T-K+1=4090.

Strategy: data-parallel across the 8 NeuronCores (one batch each).
Per core everything runs in transposed (channel, position) layout:
  - gates via bf16 matmul on TensorE (fp32 PSUM accumulation)
  - tanh on ScalarE (fp32-accurate spline), output bf16 to SBUF
  - 7-tap multiply/accumulate on VectorE in bf16
Loop order (lq, k, dc, cc, lt) with DMA loads issued in consumption
order, so the tensor engine starts after ~1MB has landed and never
starves: the k=0 sweep over all dc needs only w[k=0] plus the first
position blocks of x, and each later k-sweep adds one 0.5MB w slice.
Host side transposes/casts inputs (part of sharding) and transposes the
(C, L) bf16 per-core result back to the (B, L, C) fp32 output.
"""

import numpy as np
import ml_dtypes

import sys
for _p in ("/opt/trn_rl_repo", "/root/.axon_site/_ro/trn_rl_repo"):
    if _p not in sys.path:
        sys.path.append(_p)

B, T, C, K = 8, 4096, 512, 7
L = T - K + 1  # 4090
NCORES = 8
P = 128           # partitions
DC = C // P       # 4 channel chunks
NL = 512          # l-tile (one PSUM bank of fp32)
NLT = (L + NL - 1) // NL  # 8 l-tiles, last ragged (506)
QUAD = 4          # l-tiles per group
NQ = NLT // QUAD  # 2 groups
XBLK = 512        # x load chunk (columns)

_cache = {}


def _build():
    import concourse.bass as bass  # noqa: F401
    import concourse.mybir as mybir
    import concourse.tile as tile
    from concourse import bacc

    bf16 = mybir.dt.bfloat16
    f32 = mybir.dt.float32
    Tanh = mybir.ActivationFunctionType.Tanh

    nc = bacc.Bacc("TRN2", target_bir_lowering=False, debug=False,
                   num_devices=NCORES)

    xT_d = nc.dram_tensor("xT", [C, T], bf16, kind="ExternalInput")
    wT_d = nc.dram_tensor("wT", [K, C, C], bf16, kind="ExternalInput")
    outT_d = nc.dram_tensor("outT", [C, L], bf16, kind="ExternalOutput")

    with tile.TileContext(nc) as tc:
        with (
            tc.tile_pool(name="wpool", bufs=1) as wpool,
            tc.tile_pool(name="xpool", bufs=1) as xpool,
            tc.tile_pool(name="gpool", bufs=6) as gpool,
            tc.tile_pool(name="apool", bufs=8) as apool,
            tc.tile_pool(name="ppool", bufs=3) as ppool,
            tc.tile_pool(name="psum", bufs=8, space="PSUM") as psum_pool,
        ):
            # w_sb[cc][c_in, k, d] = weights[d, cc*128+c_in, k]
            w_sb = [wpool.tile([P, K, C], bf16, name=f"w_{cc}")
                    for cc in range(DC)]
            xT_sb = [xpool.tile([P, T], bf16, name=f"xt_{cc}")
                     for cc in range(DC)]

            def load_w(k):
                for cc in range(DC):
                    nc.sync.dma_start(w_sb[cc][:, k, :],
                                      wT_d.ap()[k, cc * P:(cc + 1) * P, :])

            def load_x(blk):
                c0 = blk * XBLK
                for cc in range(DC):
                    nc.sync.dma_start(
                        xT_sb[cc][:, c0:c0 + XBLK],
                        xT_d.ap()[cc * P:(cc + 1) * P, c0:c0 + XBLK])

            # DMA issue order == consumption order.
            load_w(0)
            for blk in range(5):       # cols 0..2559 cover quad 0 (+halo)
                load_x(blk)
            for k in range(1, K):
                load_w(k)
            for blk in range(5, T // XBLK):
                load_x(blk)

            for lq in range(NQ):
                q0 = lq * QUAD * NL               # first output col
                qn = min(QUAD * NL, L - q0)       # 2048 / 2042
                acc = [None] * DC
                for k in range(K):
                    for dc in range(DC):
                        ps = [psum_pool.tile([P, NL], f32, tag="ps",
                                             name=f"ps_{lq}_{k}_{dc}_{i}")
                              for i in range(QUAD)]
                        for cc in range(DC):
                            lhsT = w_sb[cc][:, k, dc * P:(dc + 1) * P]
                            for i in range(QUAD):
                                l0 = q0 + i * NL
                                nl = min(NL, L - l0)
                                nc.tensor.matmul(
                                    ps[i][:, :nl],
                                    lhsT,
                                    xT_sb[cc][:, l0 + K - 1: l0 + K - 1 + nl],
                                    start=(cc == 0),
                                    stop=(cc == DC - 1),
                                )
                        g = gpool.tile([P, QUAD * NL], bf16, tag="g",
                                       name=f"g_{lq}_{k}_{dc}")
                        for i in range(QUAD):
                            l0 = q0 + i * NL
                            nl = min(NL, L - l0)
                            nc.scalar.activation(
                                g[:, i * NL:i * NL + nl], ps[i][:, :nl], Tanh)
                        xu = xT_sb[dc][:, q0 + k:q0 + k + qn]
                        if acc[dc] is None:
                            a0 = apool.tile([P, QUAD * NL], bf16, tag="acc",
                                            name=f"acc_{lq}_{k}_{dc}")
                            nc.vector.tensor_mul(a0[:, :qn], g[:, :qn], xu)
                            acc[dc] = a0
                        else:
                            prod = ppool.tile([P, QUAD * NL], bf16,
                                              tag="prod",
                                              name=f"prod_{lq}_{k}_{dc}")
                            nc.vector.tensor_mul(prod[:, :qn], g[:, :qn], xu)
                            nxt = apool.tile([P, QUAD * NL], bf16, tag="acc",
                                             name=f"accn_{lq}_{k}_{dc}")
                            nc.vector.tensor_add(nxt[:, :qn], acc[dc][:, :qn],
                                                 prod[:, :qn])
                            acc[dc] = nxt
                for dc in range(DC):
                    nc.sync.dma_start(
                        outT_d.ap()[dc * P:(dc + 1) * P, q0:q0 + qn],
                        acc[dc][:, :qn])

    nc.compile()
    return nc


def _prep_inputs(x, weights):
    bf = ml_dtypes.bfloat16
    wT = np.transpose(weights, (2, 1, 0)).astype(bf)  # (K, C, D)
    wT = np.ascontiguousarray(wT)
    in_maps = []
    for b in range(B):
        xT = x[b].T.astype(bf)  # (C, T) contiguous
        in_maps.append({"xT": xT, "wT": wT})
    return in_maps


def kernel(x, weights):
    x = np.asarray(x, dtype=np.float32)
    weights = np.asarray(weights, dtype=np.float32)
    assert x.shape == (B, T, C) and weights.shape == (C, C, K)

    from concourse.bass_utils import run_bass_kernel_spmd

    if "nc" not in _cache:
        _cache["nc"] = _build()
    nc = _cache["nc"]

    in_maps = _prep_inputs(x, weights)
    res = run_bass_kernel_spmd(nc, in_maps, list(range(NCORES)))

    out = np.empty((B, L, C), dtype=np.float32)
    for b in range(B):
        out[b] = res.results[b]["outT"].astype(np.float32).T
    return out


if __name__ == "__main__":
    rng = np.random.default_rng(0)
    x = rng.standard_normal((B, T, C), dtype=np.float32)
    w = (rng.standard_normal((C, C, K), dtype=np.float32)
         / np.sqrt(np.float32(C * K)))
    out = kernel(x, w)
    print("out", out.shape, out.dtype, float(np.abs(out).max()))
